# revision 7
# baseline (speedup 1.0000x reference)
"""HardBootstrappingLoss Trainium2 Bass kernel.

Math (per row i of y_pred [B, C], y [B]):
  p = softmax(y_pred[i]);  top-3 values/indices of p == top-3 of raw logits
  fy = p[i, y_i];  s_i = fy < 0.02
  w = top3 probs renormalized;  soft target scattered at top3 idx (s=1)
  hard target one_hot(y) (s=0)
  bootstrap = mean_i( s_i * (logZ - sum_k w_k v_k) + (1-s_i) * (logZ - g_i) )
  (logits ~ N(0,1) so exp() needs no max-shift: Z = sum exp(x), logp = x - logZ)

Outputs mirror the reference tuple:
  (bootstrap, s, s*argmax|-1, s*idx2|-1, s*idx3|-1, sum(1-s), s*p_second)

Device strategy (8 cores, data parallel over batch):
  Each core gets 512 rows. Rows map to SBUF partitions (128 rows x 4 tiles).
  Stream the class dim in chunks; per chunk compute per-256-block maxes (DVE)
  and exp+row-sum (ACT accum). Then pick the top-3 blocks per row via
  max/max_index on the block maxes (stable multiset semantics match
  jax.lax.top_k tie handling), indirect-DMA re-gather just those 3 blocks
  (+ the label element), and extract exact top-3 values/columns from the
  768-wide gathered buffer. Everything else is tiny [128,k] math.
"""

import os
import sys

import numpy as np

for _p in ("/opt/trn_rl_repo", "/root/.axon_site/_ro/trn_rl_repo"):
    if os.path.isdir(_p) and _p not in sys.path:
        sys.path.append(_p)

import concourse.bass as bass
import concourse.mybir as mybir
from concourse.bass import IndirectOffsetOnAxis
from concourse.tile import TileContext

F32 = mybir.dt.float32
BF16 = mybir.dt.bfloat16
I32 = mybir.dt.int32
U32 = mybir.dt.uint32

P = 128
NEG = -1.0e30
LN002 = float(np.log(np.float64(0.02)))


class Cfg:
    def __init__(self, rows=512, C=50257, F=8192, BS=256):
        self.rows, self.C, self.F, self.BS = rows, C, F, BS
        assert rows % P == 0 and F % BS == 0
        self.NT = rows // P
        n_full = C // F
        tail = C - n_full * F
        chunks = []  # (c0, w, wpad, blk0, nblk)
        blk0 = 0
        for j in range(n_full):
            chunks.append((j * F, F, F, blk0, F // BS))
            blk0 += F // BS
        if tail:
            tb = -(-tail // BS)
            chunks.append((n_full * F, tail, tb * BS, blk0, tb))
            blk0 += tb
        self.chunks = chunks
        self.NBLK = blk0
        self.CLAMP = C - BS
        assert 8 <= self.NBLK <= 16384


def build_kernel(tc, outs, ins, cfg: Cfg):
    from contextlib import ExitStack

    nc = tc.nc
    C, F, BS, NT = cfg.C, cfg.F, cfg.BS, cfg.NT
    NCH = len(cfg.chunks)
    yp, yy = ins["yp"], ins["yy"]
    oint, om9, oloss = outs["oint"], outs["om9"], outs["oloss"]

    ctx = ExitStack()
    cpool = ctx.enter_context(tc.tile_pool(name="const", bufs=1))
    chpool = ctx.enter_context(tc.tile_pool(name="chunks", bufs=3))
    epool = ctx.enter_context(tc.tile_pool(name="esc", bufs=2))
    spool = ctx.enter_context(tc.tile_pool(name="small", bufs=2))
    ppool = ctx.enter_context(tc.tile_pool(name="psum", bufs=1, space="PSUM"))

    # constants / whole-kernel accumulators
    iota_i = cpool.tile([P, BS], I32)
    nc.gpsimd.iota(iota_i[:], pattern=[[1, BS]], base=0, channel_multiplier=0)
    iota_f = cpool.tile([P, BS], F32)
    nc.vector.tensor_copy(iota_f[:], iota_i[:])
    ones = cpool.tile([P, 1], F32)
    nc.vector.memset(ones[:], 1.0)
    int_out = cpool.tile([P, NT * 4], I32)
    m9_out = cpool.tile([P, NT], F32)
    lbuf = cpool.tile([P, NT], F32)

    for t in range(NT):
        rs = slice(t * P, (t + 1) * P)
        sacc = spool.tile([P, NCH], F32)
        bm = spool.tile([P, cfg.NBLK], F32)

        # ---- streaming pass: block maxes + exp row-sums ----
        for j, (c0, w, wpad, blk0, nblk) in enumerate(cfg.chunks):
            ch = chpool.tile([P, wpad], F32, tag="ch")
            nc.sync.dma_start(out=ch[:, :w], in_=yp[rs, c0 : c0 + w])
            if wpad > w:
                nc.gpsimd.memset(ch[:, w:wpad], NEG)
            nc.vector.reduce_max(
                out=bm[:, blk0 : blk0 + nblk],
                in_=ch[:].rearrange("p (b s) -> p b s", s=BS),
                axis=mybir.AxisListType.X,
            )
            esc = epool.tile([P, w], BF16, tag="esc")
            nc.scalar.activation(
                esc[:], ch[:, :w], mybir.ActivationFunctionType.Exp,
                accum_out=sacc[:, j : j + 1],
            )

        # ---- pick top-3 blocks per row ----
        bm8 = spool.tile([P, 8], F32)
        nc.vector.max(out=bm8[:], in_=bm[:])
        bidx = spool.tile([P, 8], U32)
        nc.vector.max_index(out=bidx[:], in_max=bm8[:], in_values=bm[:])
        bf = spool.tile([P, 3], F32)
        nc.vector.tensor_copy(bf[:], bidx[:, :3])
        # sort the 3 block ids ascending (lo, mid, hi)
        bsort = spool.tile([P, 3], F32)
        tmp = spool.tile([P, 2], F32)
        nc.vector.tensor_tensor(tmp[:, 0:1], bf[:, 0:1], bf[:, 1:2], op=mybir.AluOpType.min)
        nc.vector.tensor_tensor(bsort[:, 0:1], tmp[:, 0:1], bf[:, 2:3], op=mybir.AluOpType.min)
        nc.vector.tensor_tensor(tmp[:, 1:2], bf[:, 0:1], bf[:, 1:2], op=mybir.AluOpType.max)
        nc.vector.tensor_tensor(bsort[:, 2:3], tmp[:, 1:2], bf[:, 2:3], op=mybir.AluOpType.max)
        tsum = spool.tile([P, 1], F32)
        nc.vector.tensor_tensor(tsum[:], bf[:, 0:1], bf[:, 1:2], op=mybir.AluOpType.add)
        nc.vector.tensor_tensor(tsum[:], tsum[:], bf[:, 2:3], op=mybir.AluOpType.add)
        nc.vector.tensor_tensor(tsum[:], tsum[:], bsort[:, 0:1], op=mybir.AluOpType.subtract)
        nc.vector.tensor_tensor(bsort[:, 1:2], tsum[:], bsort[:, 2:3], op=mybir.AluOpType.subtract)
        # block start columns, clamped so a gather never crosses a row end
        starts = spool.tile([P, 3], F32)
        nc.vector.tensor_scalar(
            starts[:], bsort[:], float(BS), float(cfg.CLAMP),
            op0=mybir.AluOpType.mult, op1=mybir.AluOpType.min,
        )
        starts_i = spool.tile([P, 3], I32)
        nc.vector.tensor_copy(starts_i[:], starts[:])
        rowbase = spool.tile([P, 1], I32)
        nc.gpsimd.iota(rowbase[:], pattern=[[0, 1]], base=t * P * C, channel_multiplier=C)
        # NB: integer adds must run on GPSIMD (Q7) — the DVE ALU routes int32
        # through f32 and rounds offsets >= 2^24.
        offs = spool.tile([P, 3], I32)
        nc.gpsimd.tensor_tensor(
            offs[:], starts_i[:], rowbase[:].to_broadcast([P, 3]), op=mybir.AluOpType.add
        )
        yv = spool.tile([P, 1], I32)
        nc.sync.dma_start(out=yv[:], in_=yy[rs, None])
        goff = spool.tile([P, 1], I32)
        nc.gpsimd.tensor_tensor(goff[:], yv[:], rowbase[:], op=mybir.AluOpType.add)

        # ---- indirect re-gather of the 3 candidate blocks + label logit ----
        gbuf = spool.tile([P, 3 * BS], F32)
        for k in range(3):
            nc.gpsimd.indirect_dma_start(
                out=gbuf[:, k * BS : (k + 1) * BS], out_offset=None,
                in_=yp[:, :], in_offset=IndirectOffsetOnAxis(ap=offs[:, k : k + 1], axis=1),
            )
        gval = spool.tile([P, 1], F32)
        nc.gpsimd.indirect_dma_start(
            out=gval[:], out_offset=None,
            in_=yp[:, :], in_offset=IndirectOffsetOnAxis(ap=goff[:], axis=1),
        )

        # column map of gathered elements; mask duplicate columns in region 2
        # (only the clamped tail block can overlap its predecessor)
        colmap = spool.tile([P, 3 * BS], F32)
        for k in range(3):
            nc.vector.tensor_scalar_add(
                colmap[:, k * BS : (k + 1) * BS], iota_f[:], starts[:, k : k + 1]
            )
        thr = spool.tile([P, 1], F32)
        nc.vector.tensor_scalar_add(thr[:], starts[:, 1:2], float(BS))
        dpen = spool.tile([P, BS], F32)
        nc.vector.tensor_scalar(
            dpen[:], colmap[:, 2 * BS :], thr[:], NEG,
            op0=mybir.AluOpType.is_lt, op1=mybir.AluOpType.mult,
        )
        nc.vector.tensor_tensor(
            gbuf[:, 2 * BS :], gbuf[:, 2 * BS :], dpen[:], op=mybir.AluOpType.add
        )

        # ---- exact top-3 values + positions (stable ties) ----
        v8 = spool.tile([P, 8], F32)
        nc.vector.max(out=v8[:], in_=gbuf[:])
        pos = spool.tile([P, 8], U32)
        nc.vector.max_index(out=pos[:], in_max=v8[:], in_values=gbuf[:])
        posf = spool.tile([P, 3], F32)
        nc.vector.tensor_copy(posf[:], pos[:, :3])
        # map gbuf position -> global column:
        # col = posf + s0 + (posf>=BS)*(s1-s0-BS) + (posf>=2BS)*(s2-s1-BS)
        d01 = spool.tile([P, 1], F32)
        nc.vector.tensor_tensor(d01[:], starts[:, 1:2], starts[:, 0:1], op=mybir.AluOpType.subtract)
        nc.vector.tensor_scalar_add(d01[:], d01[:], -float(BS))
        d12 = spool.tile([P, 1], F32)
        nc.vector.tensor_tensor(d12[:], starts[:, 2:3], starts[:, 1:2], op=mybir.AluOpType.subtract)
        nc.vector.tensor_scalar_add(d12[:], d12[:], -float(BS))
        q1 = spool.tile([P, 3], F32)
        nc.vector.tensor_scalar(
            q1[:], posf[:], float(BS), d01[:], op0=mybir.AluOpType.is_ge, op1=mybir.AluOpType.mult
        )
        q2 = spool.tile([P, 3], F32)
        nc.vector.tensor_scalar(
            q2[:], posf[:], float(2 * BS), d12[:], op0=mybir.AluOpType.is_ge, op1=mybir.AluOpType.mult
        )
        col = spool.tile([P, 3], F32)
        nc.vector.tensor_tensor(col[:], posf[:], q1[:], op=mybir.AluOpType.add)
        nc.vector.tensor_tensor(col[:], col[:], q2[:], op=mybir.AluOpType.add)
        nc.vector.tensor_scalar_add(col[:], col[:], starts[:, 0:1])

        # ---- row stats: Z, logZ, noisy mask s ----
        zrow = spool.tile([P, 1], F32)
        nc.vector.reduce_sum(out=zrow[:], in_=sacc[:], axis=mybir.AxisListType.X)
        logz = spool.tile([P, 1], F32)
        nc.scalar.activation(logz[:], zrow[:], mybir.ActivationFunctionType.Ln)
        svec = spool.tile([P, 1], F32)
        nc.vector.tensor_scalar(
            svec[:], gval[:], logz[:], LN002,
            op0=mybir.AluOpType.subtract, op1=mybir.AluOpType.is_lt,
        )

        # ---- soft-target stats from top-3 values ----
        e3 = spool.tile([P, 3], F32)
        nc.scalar.activation(e3[:], v8[:, :3], mybir.ActivationFunctionType.Exp)
        wsum = spool.tile([P, 1], F32)
        nc.vector.reduce_sum(out=wsum[:], in_=e3[:], axis=mybir.AxisListType.X)
        ev = spool.tile([P, 3], F32)
        nc.vector.tensor_tensor(ev[:], e3[:], v8[:, :3], op=mybir.AluOpType.mult)
        wv = spool.tile([P, 1], F32)
        nc.vector.reduce_sum(out=wv[:], in_=ev[:], axis=mybir.AxisListType.X)
        rw = spool.tile([P, 1], F32)
        nc.vector.reciprocal(rw[:], wsum[:])
        wvn = spool.tile([P, 1], F32)
        nc.vector.tensor_tensor(wvn[:], wv[:], rw[:], op=mybir.AluOpType.mult)
        rz = spool.tile([P, 1], F32)
        nc.vector.reciprocal(rz[:], zrow[:])
        p1 = spool.tile([P, 1], F32)
        nc.vector.tensor_tensor(p1[:], e3[:, 1:2], rz[:], op=mybir.AluOpType.mult)

        # ---- per-row loss and outputs ----
        sel = spool.tile([P, 1], F32)
        nc.vector.tensor_tensor(sel[:], wvn[:], gval[:], op=mybir.AluOpType.subtract)
        nc.vector.tensor_tensor(sel[:], sel[:], svec[:], op=mybir.AluOpType.mult)
        nc.vector.tensor_tensor(sel[:], sel[:], gval[:], op=mybir.AluOpType.add)
        nc.vector.tensor_tensor(lbuf[:, t : t + 1], logz[:], sel[:], op=mybir.AluOpType.subtract)
        nc.vector.tensor_tensor(m9_out[:, t : t + 1], svec[:], p1[:], op=mybir.AluOpType.mult)
        idxf = spool.tile([P, 3], F32)
        nc.vector.tensor_scalar(
            idxf[:], col[:], 1.0, svec[:], op0=mybir.AluOpType.add, op1=mybir.AluOpType.mult
        )
        nc.vector.tensor_scalar_add(idxf[:], idxf[:], -1.0)
        nc.vector.tensor_copy(int_out[:, t * 4 + 1 : t * 4 + 4], idxf[:])
        nc.vector.tensor_copy(int_out[:, t * 4 : t * 4 + 1], svec[:])

    # ---- per-core partial loss (cross-partition sum via PE) ----
    psum = ppool.tile([1, NT], F32)
    nc.tensor.matmul(out=psum[:], lhsT=ones[:], rhs=lbuf[:], start=True, stop=True)
    lsum = spool.tile([1, 1], F32)
    nc.vector.reduce_sum(out=lsum[:], in_=psum[:], axis=mybir.AxisListType.X)
    nc.sync.dma_start(out=oloss[:, :], in_=lsum[:])
    nc.sync.dma_start(out=oint[:, :], in_=int_out[:])
    nc.sync.dma_start(out=om9[:, :], in_=m9_out[:])
    ctx.close()


def build_bass(cfg: Cfg, num_devices=8, compile=True):
    import concourse.bacc as bacc

    nc = bacc.Bacc("TRN2", target_bir_lowering=False, debug=False,
                   num_devices=num_devices)
    ins = {
        "yp": nc.dram_tensor("yp", [cfg.rows, cfg.C], F32, kind="ExternalInput").ap(),
        "yy": nc.dram_tensor("yy", [cfg.rows], I32, kind="ExternalInput").ap(),
    }
    outs = {
        "oint": nc.dram_tensor("oint", [P, cfg.NT * 4], I32, kind="ExternalOutput").ap(),
        "om9": nc.dram_tensor("om9", [P, cfg.NT], F32, kind="ExternalOutput").ap(),
        "oloss": nc.dram_tensor("oloss", [1, 1], F32, kind="ExternalOutput").ap(),
    }
    with TileContext(nc) as tc:
        build_kernel(tc, outs, ins, cfg)
    if compile:
        nc.compile()
    return nc


_CACHE = {}


def _get_nc():
    if "nc" not in _CACHE:
        _CACHE["nc"] = build_bass(Cfg())
    return _CACHE["nc"]


def kernel(y_pred: np.ndarray, y: np.ndarray):
    from concourse.bass_utils import run_bass_kernel_spmd

    B, C = 4096, 50257
    assert y_pred.shape == (B, C) and y.shape == (B,)
    y_pred = np.ascontiguousarray(y_pred, dtype=np.float32)
    y = np.ascontiguousarray(y, dtype=np.int32)
    ncores, rows = 8, 512
    nc = _get_nc()
    in_maps = [
        {"yp": y_pred[c * rows : (c + 1) * rows], "yy": y[c * rows : (c + 1) * rows]}
        for c in range(ncores)
    ]
    res = run_bass_kernel_spmd(nc, in_maps, core_ids=list(range(ncores)),
                               trace=bool(int(os.environ.get("KERNEL_TRACE", "0"))))
    _CACHE["last_results"] = res

    s = np.empty(B, np.int32)
    zz = np.empty(B, np.int32)
    i9 = np.empty(B, np.int32)
    i8 = np.empty(B, np.int32)
    m9 = np.empty(B, np.float32)
    loss = 0.0
    for c, r in enumerate(res.results):
        oint, om9 = r["oint"], r["om9"]
        loss += float(r["oloss"][0, 0])
        for t in range(4):
            rows_sl = slice(c * rows + t * P, c * rows + (t + 1) * P)
            s[rows_sl] = oint[:, t * 4]
            zz[rows_sl] = oint[:, t * 4 + 1]
            i9[rows_sl] = oint[:, t * 4 + 2]
            i8[rows_sl] = oint[:, t * 4 + 3]
            m9[rows_sl] = om9[:, t]
    bootstrap = np.float32(loss / B)
    count = np.int32(B - int(s.sum()))
    return (bootstrap, s, zz, i9, i8, count, m9)
